# revision 1
# baseline (speedup 1.0000x reference)
"""Multi-head GAT layer (GATConv + BatchNorm + ELU) on 8 trn2 NeuronCores.

Dst-sharded graph parallelism, gather-free edition:
  - Launch A (tiny): per-node a_src/a_dst tables (x @ [Ws|Wd]), the per-head
    edge coefficient we[h], partial sums of edge_attr (for the self-loop
    fill value).
  - Host: adds self-loop edges, buckets edges per dst tile, expands
    per-edge streams BY INDEXING ONLY: xe = xT[:, src_e] (bf16), packed
    per-edge scalars [a_src | a_dst | ea | dstrel] (bf16).  No on-device
    gather: the source features arrive as a sequential full-bandwidth
    stream, eliminating the gpsimd descriptor-generation bottleneck.
  - Launch B (main): per dst tile, per 128-edge chunk:
      h_e   = xe_chunk @ W                       (PE, bf16 -> PSUM f32)
      alpha = asrc + adst + ea*we; w = exp(leaky_relu(alpha))  (vec/scalar)
      mb    = [h_e * w_per_head | w]             (vec, bf16)
      acc  += onehot(dstrel)^T @ mb              (PE scatter-add in PSUM)
    then normalizes by the per-dst denominator, emits bf16 out_pre rows and
    accumulates per-channel sum/sumsq for batchnorm via a ones-matmul.
  - Host: sums the 8 partial stat vectors (glue).
  - Launch C (tiny): batchnorm + ELU as a per-channel affine in transposed
    layout (bf16 in, f32 out).

All floating-point math runs on device; the host only shards, sorts,
expands by indexing, converts dtypes, and adds a handful of partial
scalars.
"""
import os

import numpy as np
import ml_dtypes

import concourse.bacc as bacc
import concourse.mybir as mybir
import concourse.tile as tile
from concourse import bass_utils
from concourse.vector_clock import ScopedClock

F32 = mybir.dt.float32
BF16 = mybir.dt.bfloat16
NPBF = ml_dtypes.bfloat16
NEG_SLOPE = 0.2
BN_EPS = 1e-5
NCORES = 8
P = 128

LAST_RESULTS = []  # BassKernelResults of the last kernel() call (A, B, C)


def _patch_tile_drain():
    """This walrus build rejects multiple sem waits on the Tile tail Drain
    ("Too many sync wait commands"); move each wait onto its own NOP."""
    if getattr(tile.TileContext, "_gat_drain_patched", False):
        return

    def _drain_and_barrier(self, tick_clock, wait_clock):
        nc = self.nc
        drain_inst = nc.sync.drain()
        wait_clock.add_sem_waits(
            drain_inst.ins, ScopedClock({None: tick_clock.global_clock})
        )
        si = drain_inst.ins.sync_info
        if si is not None and si.on_wait:
            waits = list(si.on_wait)
            drain_inst.ins.sync_info = mybir.SyncInfo(
                on_wait=[], on_update=list(si.on_update)
            )
            for w in waits:
                n = nc.sync.nop(nofuse=True, hint="drain_wait")
                n.ins.sync_info = mybir.SyncInfo(on_wait=[w], on_update=[])
        nc.all_engine_barrier()
        popped = nc._tile_sem_poison_stack.pop()
        assert popped is self._sem_poison
        nc.clear_and_free_semaphores(list(self.sems.allocated().values()))
        nc.all_engine_barrier()

    tile.TileContext._drain_and_barrier = _drain_and_barrier
    tile.TileContext._gat_drain_patched = True


def _run(nc, in_maps, out_names):
    if os.environ.get("GAT_SIM"):
        from concourse.bass_interp import CoreSim

        results = []
        for m in in_maps:
            sim = CoreSim(nc, trace=False, require_finite=False,
                          require_nnan=False)
            for k, v in m.items():
                sim.tensor(k)[:] = v
            sim.simulate()
            results.append({k: np.array(sim.tensor(k)[:]) for k in out_names})

        class R:
            pass

        r = R()
        r.results = results
        r.exec_time_ns = None
        return r
    return bass_utils.run_bass_kernel_spmd(
        nc, in_maps, core_ids=list(range(NCORES)))


def kernel(x, edge_index, edge_attr, W, W_edge, att_src, att_dst, att_edge,
           bias, gamma, beta):
    _patch_tile_drain()
    global LAST_RESULTS
    LAST_RESULTS = []

    x = np.asarray(x, dtype=np.float32)
    edge_index = np.asarray(edge_index)
    edge_attr = np.asarray(edge_attr, dtype=np.float32)
    W = np.asarray(W, dtype=np.float32)
    W_edge = np.asarray(W_edge, dtype=np.float32)
    att_src = np.asarray(att_src, dtype=np.float32)
    att_dst = np.asarray(att_dst, dtype=np.float32)
    att_edge = np.asarray(att_edge, dtype=np.float32)
    gamma = np.asarray(gamma, dtype=np.float32)
    beta = np.asarray(beta, dtype=np.float32)

    N, IN = x.shape
    H, C = att_src.shape
    HC = H * C
    MBW = HC + H  # message row width: HC channels + H denominator slots
    E = edge_index.shape[1]
    assert IN == P and N % NCORES == 0
    SH = N // NCORES
    T = (SH + P - 1) // P
    TF = SH // P          # full tiles
    LROWS = SH - TF * P   # rows in last (partial) tile
    src_all = edge_index[0].astype(np.int64)
    dst_all = edge_index[1].astype(np.int64)
    ea_all = edge_attr[:, 0].astype(np.float32)

    xT = np.ascontiguousarray(x.T)
    xT_bf = xT.astype(NPBF)
    asrc_rep = np.tile(att_src.reshape(1, HC), (P, 1)).astype(np.float32)
    adst_rep = np.tile(att_dst.reshape(1, HC), (P, 1)).astype(np.float32)
    iota4 = np.tile(np.tile(np.arange(P, dtype=np.float32).reshape(1, P),
                            (P, 1)), (1, 4)).astype(NPBF)
    ones_col = np.ones((P, 1), dtype=np.float32)
    ones_bf = np.ones((P, 1), dtype=NPBF)

    # ------------------------------------------------------------------
    # Launch A: a_src/a_dst tables, we[h], partial edge_attr sums
    # ------------------------------------------------------------------
    EASH = ((E // NCORES) + P - 1) // P * P
    nc = bacc.Bacc("TRN2", target_bir_lowering=False, debug=False)
    d = {}
    d["xT_sh"] = nc.dram_tensor("xT_sh", [P, SH], BF16, kind="ExternalInput")
    d["W"] = nc.dram_tensor("W", [P, HC], F32, kind="ExternalInput")
    d["asrc_rep"] = nc.dram_tensor("asrc_rep", [P, HC], F32, kind="ExternalInput")
    d["adst_rep"] = nc.dram_tensor("adst_rep", [P, HC], F32, kind="ExternalInput")
    d["wedge"] = nc.dram_tensor("wedge", [1, HC], F32, kind="ExternalInput")
    d["aedge"] = nc.dram_tensor("aedge", [1, HC], F32, kind="ExternalInput")
    d["ea_sh"] = nc.dram_tensor("ea_sh", [P, EASH // P], F32, kind="ExternalInput")
    d["ones_col"] = nc.dram_tensor("ones_col", [P, 1], F32, kind="ExternalInput")
    atab_t = nc.dram_tensor("a_tab", [SH, 2 * H], F32, kind="ExternalOutput")
    we_t = nc.dram_tensor("we_out", [1, H], F32, kind="ExternalOutput")
    eas_t = nc.dram_tensor("ea_sum", [1, 1], F32, kind="ExternalOutput")

    with tile.TileContext(nc) as tc:
        with tc.tile_pool(name="sbuf", bufs=2) as pool, \
             tc.tile_pool(name="psum", bufs=2, space="PSUM") as pp:
            w_sb = pool.tile([P, HC], F32, tag="w")
            nc.sync.dma_start(out=w_sb[:], in_=d["W"].ap())
            ar_sb = pool.tile([P, HC], F32, tag="ar")
            nc.sync.dma_start(out=ar_sb[:], in_=d["asrc_rep"].ap())
            ad_sb = pool.tile([P, HC], F32, tag="ad")
            nc.sync.dma_start(out=ad_sb[:], in_=d["adst_rep"].ap())
            on_sb = pool.tile([P, 1], F32, tag="ones")
            nc.sync.dma_start(out=on_sb[:], in_=d["ones_col"].ap())
            wswd = pool.tile([P, 2 * H], F32, tag="wswd")
            tmp = pool.tile([P, HC], F32, tag="tmp")
            nc.vector.tensor_mul(tmp[:], w_sb[:], ar_sb[:])
            for h in range(H):
                nc.vector.reduce_sum(wswd[:, h:h + 1], tmp[:, h * C:(h + 1) * C],
                                     axis=mybir.AxisListType.X)
            nc.vector.tensor_mul(tmp[:], w_sb[:], ad_sb[:])
            for h in range(H):
                nc.vector.reduce_sum(wswd[:, H + h:H + h + 1],
                                     tmp[:, h * C:(h + 1) * C],
                                     axis=mybir.AxisListType.X)
            we_row = pool.tile([1, HC], F32, tag="we_row")
            nc.sync.dma_start(out=we_row[:], in_=d["wedge"].ap())
            ae_row = pool.tile([1, HC], F32, tag="ae_row")
            nc.sync.dma_start(out=ae_row[:], in_=d["aedge"].ap())
            nc.vector.tensor_mul(we_row[:], we_row[:], ae_row[:])
            we_sb = pool.tile([1, H], F32, tag="we_sb")
            for h in range(H):
                nc.vector.reduce_sum(we_sb[:, h:h + 1],
                                     we_row[:, h * C:(h + 1) * C],
                                     axis=mybir.AxisListType.X)
            nc.sync.dma_start(out=we_t.ap(), in_=we_sb[:])
            ea_sb = pool.tile([P, EASH // P], F32, tag="ea")
            nc.sync.dma_start(out=ea_sb[:], in_=d["ea_sh"].ap())
            red = pool.tile([P, 1], F32, tag="red")
            nc.vector.reduce_sum(red[:], ea_sb[:], axis=mybir.AxisListType.X)
            ps1 = pp.tile([1, 1], F32, tag="ps1")
            nc.tensor.matmul(ps1[:], lhsT=on_sb[:], rhs=red[:], start=True,
                             stop=True)
            sc = pool.tile([1, 1], F32, tag="sc")
            nc.vector.tensor_copy(sc[:], ps1[:])
            nc.sync.dma_start(out=eas_t.ap(), in_=sc[:])

            xsh = pool.tile([P, SH], BF16, tag="xsh")
            nc.sync.dma_start(out=xsh[:], in_=d["xT_sh"].ap())
            wswd_bf = pool.tile([P, 2 * H], BF16, tag="wswdb")
            nc.vector.tensor_copy(wswd_bf[:], wswd[:])
            atab_sb = pool.tile([P, T * 2 * H], F32, tag="atab")
            for t in range(T):
                rows = min(P, SH - t * P)
                ps = pp.tile([P, 2 * H], F32, tag="ps")
                nc.tensor.matmul(ps[:rows, :], lhsT=xsh[:, t * P:t * P + rows],
                                 rhs=wswd_bf[:], start=True, stop=True)
                nc.vector.tensor_copy(atab_sb[:rows, t * 2 * H:(t + 1) * 2 * H],
                                      ps[:rows, :])
            nc.sync.dma_start(
                out=atab_t.ap()[0:TF * P, :].rearrange("(t p) h -> p t h", p=P),
                in_=atab_sb[:, :TF * 2 * H].rearrange("p (t h) -> p t h",
                                                      h=2 * H))
            if LROWS:
                nc.sync.dma_start(
                    out=atab_t.ap()[TF * P:SH, :],
                    in_=atab_sb[:LROWS, TF * 2 * H:T * 2 * H])
    nc.compile()

    in_maps = []
    for c in range(NCORES):
        ea_sl = np.zeros(EASH, dtype=np.float32)
        lo, hi = c * (E // NCORES), (c + 1) * (E // NCORES)
        if c == NCORES - 1:
            hi = E
        seg = ea_all[lo:hi]
        ea_sl[:seg.shape[0]] = seg
        in_maps.append({
            "xT_sh": np.ascontiguousarray(xT_bf[:, c * SH:(c + 1) * SH]),
            "W": W, "asrc_rep": asrc_rep, "adst_rep": adst_rep,
            "wedge": W_edge.reshape(1, HC).astype(np.float32),
            "aedge": att_edge.reshape(1, HC).astype(np.float32),
            "ea_sh": np.ascontiguousarray(ea_sl.reshape(EASH // P, P).T),
            "ones_col": ones_col,
        })
    resA = _run(nc, in_maps, ["a_tab", "we_out", "ea_sum"])
    LAST_RESULTS.append(resA)

    a_tab = np.concatenate([r["a_tab"] for r in resA.results], axis=0)
    we = resA.results[0]["we_out"][0].astype(np.float32)
    ea_mean = float(sum(float(r["ea_sum"][0, 0]) for r in resA.results)) / E

    # ------------------------------------------------------------------
    # Host: edges (+self-loops) -> per-core per-dst-tile chunk slots
    # ------------------------------------------------------------------
    loops = np.arange(N, dtype=np.int64)
    src_x = np.concatenate([src_all, loops])
    dst_x = np.concatenate([dst_all, loops])
    ea_x = np.concatenate([ea_all, np.full(N, ea_mean, dtype=np.float32)])

    per_core = []
    for c in range(NCORES):
        m = (dst_x >= c * SH) & (dst_x < (c + 1) * SH)
        s, dd, ee = src_x[m], dst_x[m] - c * SH, ea_x[m]
        order = np.argsort(dd, kind="stable")
        s, dd, ee = s[order], dd[order], ee[order]
        tb = dd // P  # tile of each edge (sorted, so contiguous runs)
        bounds = np.searchsorted(tb, np.arange(T + 1))
        per_core.append((s, dd, ee, bounds))

    nch = [max(int(per_core[c][3][t + 1] - per_core[c][3][t] + P - 1) // P
               for c in range(NCORES)) for t in range(T)]
    NCH = sum(nch)
    TOTMAX = max(nch)
    offs = np.concatenate([[0], np.cumsum(nch)]).astype(np.int64)

    FLD = 2 * H + 1  # packed per-edge fields: asrc(8) adst(8) ea(1)
    core_inputs = []
    for c in range(NCORES):
        s, dd, ee, bounds = per_core[c]
        gsrc = np.zeros(NCH * P, dtype=np.int64)
        pad = np.ones(NCH * P, dtype=bool)
        drel_all = np.zeros(NCH * P, dtype=np.int64)
        for t in range(T):
            lo, hi = int(bounds[t]), int(bounds[t + 1])
            n = hi - lo
            base = int(offs[t]) * P
            gsrc[base:base + n] = s[lo:hi]
            pad[base:base + n] = False
            drel_all[base:base + n] = dd[lo:hi] - t * P
        # xe: [128 xdim, NCH*128] bf16, col (k*128+j) = xT[:, src of slot j]
        xe = xT_bf[:, gsrc]
        if pad.any():
            xe[:, pad] = NPBF(0)
        # one-hot dst-selection matrices, precomputed host-side:
        # [128 part=edge j, NCH*128], col (k*128+f) = (dstrel of slot j == f)
        soh = np.zeros((NCH, P, P), dtype=NPBF)
        real = ~pad
        slot = np.nonzero(real)[0]
        soh[slot // P, slot % P, drel_all[real]] = NPBF(1)
        soh = np.ascontiguousarray(
            soh.transpose(1, 0, 2).reshape(P, NCH * P))
        # packed per-edge scalars, field-major per tile:
        # tile block cols [off*FLD, (off+tot)*FLD) =
        #   [asrc (tot*H) | adst (tot*H) | ea (tot)]
        pk = np.zeros((P, NCH * FLD), dtype=np.float32)
        for t in range(T):
            lo, hi = int(bounds[t]), int(bounds[t + 1])
            n = hi - lo
            tot = nch[t]
            sl = slice(lo, hi)

            def expand(vals, w):
                buf = np.zeros((tot * P, w), dtype=np.float32)
                buf[:n] = vals.reshape(n, w)
                return (buf.reshape(tot, P, w).transpose(1, 0, 2)
                        .reshape(P, tot * w))

            b0 = int(offs[t]) * FLD
            pk[:, b0:b0 + tot * H] = expand(a_tab[s[sl], 0:H], H)
            pk[:, b0 + tot * H:b0 + 2 * tot * H] = expand(
                a_tab[c * SH + dd[sl], H:2 * H], H)
            pk[:, b0 + 2 * tot * H:b0 + FLD * tot] = expand(ee[sl], 1)
        core_inputs.append(dict(xe=np.ascontiguousarray(xe), soh=soh,
                                pk=pk.astype(NPBF)))

    we_tiled = np.ascontiguousarray(
        np.tile(we.reshape(1, 1, H), (P, TOTMAX, 1))
        .reshape(P, TOTMAX * H)).astype(NPBF)
    # [c,h]-major channel order: lets the msg-mul's exp-weight broadcast have
    # a packed (stride-1) innermost dim, enabling the DVE 2x mode.
    old_of_new = (np.arange(H)[None, :] * C
                  + np.arange(C)[:, None]).reshape(-1)  # new j -> old h*C+c
    W_bf = np.ascontiguousarray(W[:, old_of_new]).astype(NPBF)

    # ------------------------------------------------------------------
    # Launch B
    # ------------------------------------------------------------------
    nc = bacc.Bacc("TRN2", target_bir_lowering=False, debug=False)
    xe_t = nc.dram_tensor("xe", [P, NCH * P], BF16, kind="ExternalInput")
    soh_t = nc.dram_tensor("soh", [P, NCH * P], BF16, kind="ExternalInput")
    pk_t = nc.dram_tensor("pk", [P, NCH * FLD], BF16, kind="ExternalInput")
    W_t = nc.dram_tensor("W", [P, HC], BF16, kind="ExternalInput")
    onesb_t = nc.dram_tensor("ones_bf", [P, 1], BF16, kind="ExternalInput")
    wet_t = nc.dram_tensor("we_tiled", [P, TOTMAX * H], BF16,
                           kind="ExternalInput")
    opre_t = nc.dram_tensor("out_pre", [SH, HC], BF16, kind="ExternalOutput")
    stats_t = nc.dram_tensor("stats", [1, 2 * HC], F32, kind="ExternalOutput")

    with tile.TileContext(nc) as tc:
        with tc.tile_pool(name="const", bufs=1) as cpool:
            w_sb = cpool.tile([P, HC], BF16, tag="w")
            nc.sync.dma_start(out=w_sb[:], in_=W_t.ap())
            on_sb = cpool.tile([P, 1], BF16, tag="ones")
            nc.sync.dma_start(out=on_sb[:], in_=onesb_t.ap())
            wet_sb = cpool.tile([P, TOTMAX * H], BF16, tag="wet")
            nc.sync.dma_start(out=wet_sb[:], in_=wet_t.ap())

            with tc.tile_pool(name="xe", bufs=3) as xpool, \
                 tc.tile_pool(name="pk", bufs=3) as kpool, \
                 tc.tile_pool(name="mb", bufs=2) as mpool, \
                 tc.tile_pool(name="s", bufs=3) as spool, \
                 tc.tile_pool(name="hb", bufs=4) as hbpool, \
                 tc.tile_pool(name="fin", bufs=3) as fpool, \
                 tc.tile_pool(name="hp", bufs=4, space="PSUM") as hpp, \
                 tc.tile_pool(name="acc", bufs=2, space="PSUM") as apool, \
                 tc.tile_pool(name="stp", bufs=1, space="PSUM") as stpool:
                stats_ps = stpool.tile([1, 2 * HC], F32, tag="stats")
                for t in range(T):
                    rows = min(P, SH - t * P)
                    tot = nch[t]
                    off = int(offs[t])
                    xe_sb = xpool.tile([P, TOTMAX * P], BF16, tag="xe")
                    nc.sync.dma_start(out=xe_sb[:, :tot * P],
                                      in_=xe_t.ap()[:, off * P:(off + tot) * P])
                    s_sb = spool.tile([P, TOTMAX * P], BF16, tag="S")
                    nc.sync.dma_start(out=s_sb[:, :tot * P],
                                      in_=soh_t.ap()[:, off * P:(off + tot) * P])
                    pk_sb = kpool.tile([P, TOTMAX * FLD], BF16, tag="pk")
                    nc.sync.dma_start(
                        out=pk_sb[:, :tot * FLD],
                        in_=pk_t.ap()[:, off * FLD:(off + tot) * FLD])
                    a1 = pk_sb[:, 0:tot * H]
                    a2 = pk_sb[:, tot * H:2 * tot * H]
                    eav = pk_sb[:, 2 * tot * H:2 * tot * H + tot]
                    # alpha = asrc + adst + ea*we ; w = exp(leaky_relu(alpha))
                    nc.vector.tensor_add(a1, a1, a2)
                    nc.vector.tensor_mul(
                        a2.rearrange("p (k h) -> p k h", h=H),
                        eav.to_broadcast([P, tot, H]),
                        wet_sb[:, :tot * H].rearrange("p (k h) -> p k h", h=H))
                    nc.vector.tensor_add(a1, a1, a2)
                    nc.scalar.activation(a2, a1,
                                         mybir.ActivationFunctionType.Relu,
                                         scale=-float(1.0 - NEG_SLOPE))
                    nc.vector.tensor_add(a1, a1, a2)
                    mb = mpool.tile([P, TOTMAX * MBW], BF16, tag="mb")
                    mbv = mb[:, :tot * MBW].rearrange("p (k e) -> p k e", e=MBW)
                    nc.scalar.activation(mbv[:, :, HC:MBW],
                                         a1.rearrange("p (k h) -> p k h", h=H),
                                         mybir.ActivationFunctionType.Exp)
                    acc = apool.tile([P, MBW], F32, tag="acc")
                    for k0 in range(0, tot, 2):
                        run = min(2, tot - k0)
                        hps = hpp.tile([P, 2 * HC], F32, tag="hps")
                        for j in range(run):
                            nc.tensor.matmul(
                                hps[:, j * HC:(j + 1) * HC],
                                lhsT=xe_sb[:, (k0 + j) * P:(k0 + j + 1) * P],
                                rhs=w_sb[:], start=True, stop=True)
                        mb2 = mb[:, k0 * MBW:(k0 + run) * MBW].rearrange(
                            "p (k e) -> p k e", e=MBW)
                        ekb = mb2[:, :, HC:MBW].rearrange(
                            "p k (o h) -> p k o h", o=1).to_broadcast(
                            [P, run, C, H])
                        if (k0 // 2) % 4 != 3:
                            # scalar converts f32 PSUM -> bf16; the mul then
                            # runs all-bf16 packed (DVE 2x mode)
                            hsb = hbpool.tile([P, 2 * HC], BF16, tag="hsb")
                            nc.scalar.activation(
                                hsb[:, :run * HC], hps[:, :run * HC],
                                mybir.ActivationFunctionType.Copy)
                            nc.vector.tensor_mul(
                                mb2[:, :, 0:HC].rearrange(
                                    "p k (c h) -> p k c h", h=H),
                                hsb[:, :run * HC].rearrange(
                                    "p (k c h) -> p k c h", c=C, h=H),
                                ekb)
                        else:
                            nc.vector.tensor_mul(
                                mb2[:, :, 0:HC].rearrange(
                                    "p k (c h) -> p k c h", h=H),
                                hps[:, :run * HC].rearrange(
                                    "p (k c h) -> p k c h", c=C, h=H),
                                ekb)
                        for j in range(run):
                            k = k0 + j
                            nc.tensor.matmul(
                                acc[:], lhsT=s_sb[:, k * P:(k + 1) * P],
                                rhs=mb[:, k * MBW:(k + 1) * MBW],
                                start=(k == 0), stop=(k == tot - 1))
                    # finalize tile: normalize + stats
                    den = fpool.tile([P, H], F32, tag="den")
                    nc.vector.tensor_copy(den[:rows], acc[:rows, HC:MBW])
                    rec = fpool.tile([P, H], F32, tag="rec")
                    nc.vector.reciprocal(rec[:rows], den[:rows])
                    opsq = fpool.tile([P, 2 * HC], BF16, tag="opsq")
                    nc.vector.tensor_mul(
                        opsq[:rows, :HC].rearrange("p (c h) -> p c h", h=H),
                        acc[:rows, :HC].rearrange("p (c h) -> p c h", h=H),
                        rec[0:rows, :].rearrange("p (o h) -> p o h",
                                                 o=1).to_broadcast(
                            [rows, C, H]))
                    nc.scalar.activation(opsq[:rows, HC:], opsq[:rows, :HC],
                                         mybir.ActivationFunctionType.Square)
                    nc.tensor.matmul(stats_ps[:, :], lhsT=on_sb[:rows, :],
                                     rhs=opsq[:rows, :], start=(t == 0),
                                     stop=(t == T - 1))
                    nc.sync.dma_start(out=opre_t.ap()[t * P:t * P + rows, :],
                                      in_=opsq[:rows, :HC])
                st_sb = fpool.tile([1, 2 * HC], F32, tag="stsb")
                nc.vector.tensor_copy(st_sb[:], stats_ps[:])
                nc.sync.dma_start(out=stats_t.ap(), in_=st_sb[:])
    nc.compile()

    in_maps = []
    for c in range(NCORES):
        ci = core_inputs[c]
        in_maps.append({
            "xe": ci["xe"], "soh": ci["soh"], "pk": ci["pk"], "W": W_bf,
            "ones_bf": ones_bf, "we_tiled": we_tiled,
        })
    resB = _run(nc, in_maps, ["out_pre", "stats"])
    LAST_RESULTS.append(resB)

    out_pre = np.concatenate([np.asarray(r["out_pre"])
                              for r in resB.results], axis=0)
    stats = np.stack([np.asarray(r["stats"][0], dtype=np.float64)
                      for r in resB.results]).sum(axis=0).astype(np.float32)
    sums_col = np.ascontiguousarray(
        np.stack([stats[:HC], stats[HC:]], axis=1))  # [HC, 2]

    # ------------------------------------------------------------------
    # Launch C: batchnorm + ELU (transposed layout)
    # ------------------------------------------------------------------
    opT = np.ascontiguousarray(out_pre.reshape(NCORES, SH, HC)
                               .transpose(0, 2, 1))  # [8, HC, SH] bf16
    nc = bacc.Bacc("TRN2", target_bir_lowering=False, debug=False)
    opT_t = nc.dram_tensor("opT", [HC, SH], BF16, kind="ExternalInput")
    sums_t = nc.dram_tensor("sums_col", [HC, 2], F32, kind="ExternalInput")
    gam_t = nc.dram_tensor("gamma_col", [HC, 1], F32, kind="ExternalInput")
    bet_t = nc.dram_tensor("beta_col", [HC, 1], F32, kind="ExternalInput")
    outT_t = nc.dram_tensor("outT", [HC, SH], F32, kind="ExternalOutput")

    CT = HC // P
    with tile.TileContext(nc) as tc:
        with tc.tile_pool(name="sbuf", bufs=2) as pool:
            for ct in range(CT):
                sm = pool.tile([P, 2], F32, tag="sm")
                nc.sync.dma_start(out=sm[:], in_=sums_t.ap()[ct * P:(ct + 1) * P, :])
                gm = pool.tile([P, 1], F32, tag="gm")
                nc.sync.dma_start(out=gm[:], in_=gam_t.ap()[ct * P:(ct + 1) * P, :])
                bt = pool.tile([P, 1], F32, tag="bt")
                nc.sync.dma_start(out=bt[:], in_=bet_t.ap()[ct * P:(ct + 1) * P, :])
                mean = pool.tile([P, 1], F32, tag="mean")
                nc.vector.tensor_scalar_mul(mean[:], sm[:, 0:1], 1.0 / N)
                ex2 = pool.tile([P, 1], F32, tag="ex2")
                nc.vector.tensor_scalar_mul(ex2[:], sm[:, 1:2], 1.0 / N)
                msq = pool.tile([P, 1], F32, tag="msq")
                nc.vector.tensor_mul(msq[:], mean[:], mean[:])
                var = pool.tile([P, 1], F32, tag="var")
                nc.vector.tensor_sub(var[:], ex2[:], msq[:])
                nc.vector.tensor_scalar_add(var[:], var[:], float(BN_EPS))
                sd = pool.tile([P, 1], F32, tag="sd")
                nc.scalar.activation(sd[:], var[:],
                                     mybir.ActivationFunctionType.Sqrt)
                inv = pool.tile([P, 1], F32, tag="inv")
                nc.vector.reciprocal(inv[:], sd[:])
                scl = pool.tile([P, 1], F32, tag="scl")
                nc.vector.tensor_mul(scl[:], inv[:], gm[:])
                sh1 = pool.tile([P, 1], F32, tag="sh1")
                nc.vector.tensor_mul(sh1[:], mean[:], scl[:])
                shf = pool.tile([P, 1], F32, tag="shf")
                nc.vector.tensor_sub(shf[:], bt[:], sh1[:])
                CW = SH // 2
                for cs in range(2):
                    c0 = cs * CW
                    xt_ = pool.tile([P, CW], BF16, tag="xt")
                    nc.sync.dma_start(
                        out=xt_[:],
                        in_=opT_t.ap()[ct * P:(ct + 1) * P, c0:c0 + CW])
                    y = pool.tile([P, CW], F32, tag="y")
                    nc.scalar.activation(y[:], xt_[:],
                                         mybir.ActivationFunctionType.Identity,
                                         bias=shf[:], scale=scl[:])
                    r = pool.tile([P, CW], F32, tag="r")
                    nc.vector.tensor_scalar_max(r[:], y[:], 0.0)
                    yneg = pool.tile([P, CW], F32, tag="yneg")
                    nc.vector.tensor_sub(yneg[:], y[:], r[:])
                    e = pool.tile([P, CW], F32, tag="e")
                    nc.scalar.activation(e[:], yneg[:],
                                         mybir.ActivationFunctionType.Exp)
                    nc.vector.tensor_scalar_add(r[:], r[:], -1.0)
                    nc.vector.tensor_add(r[:], r[:], e[:])
                    nc.sync.dma_start(
                        out=outT_t.ap()[ct * P:(ct + 1) * P, c0:c0 + CW],
                        in_=r[:])
    nc.compile()

    in_maps = [{
        "opT": np.ascontiguousarray(opT[c]),
        "sums_col": sums_col,
        "gamma_col": gamma[old_of_new].reshape(HC, 1),
        "beta_col": beta[old_of_new].reshape(HC, 1),
    } for c in range(NCORES)]
    resC = _run(nc, in_maps, ["outT"])
    LAST_RESULTS.append(resC)

    outp = np.concatenate(
        [np.asarray(r["outT"]).T for r in resC.results], axis=0)  # [N, HC]
    out = np.empty_like(outp)
    out[:, old_of_new] = outp  # undo the [c,h] channel permutation
    return np.ascontiguousarray(out.astype(np.float32))



# revision 3
# speedup vs baseline: 1.0720x; 1.0720x over previous
"""Multi-head GAT layer (GATConv + BatchNorm + ELU) on 8 trn2 NeuronCores.

Dst-sharded graph parallelism, v2 (mixed he/xe streaming):
  - Launch A: per-node h = x@W (bf16, [c-major,h-minor] channel order),
    a_src/a_dst tables, per-head edge coefficient we[h], partial sums of
    edge_attr (for the self-loop fill value).
  - Host: adds self-loops, buckets edges per 128-dst tile, expands per-edge
    streams BY INDEXING ONLY: a fraction R_HE of each tile's 128-edge
    chunks stream the PRE-PROJECTED source features he = h[src] (bf16,
    256 wide -> no PE projection matmul needed), the rest stream raw
    source features xe = xT[:, src] (bf16, 128 wide -> cheaper DMA but
    a PE projection per chunk).  One-hot dst-scatter matrices are sent
    in fp8 (exact for 0/1, half the DMA, 4x faster weight load).
  - Launch B: per dst tile, per 128-edge chunk:
      w     = exp(leaky_relu(asrc+adst+ea*we))        (vec/scalar, per tile)
      mb    = [he*w | w]  or  [(xe@W)*w | w]          (one fused DVE mul)
      acc  += onehot_fp8^T @ mb                       (PE scatter-add, PSUM)
    then normalizes by the per-dst denominator, emits bf16 out_pre rows and
    per-channel sum/sumsq stats via a ones-matmul.
  - Host: sums the 8 partial stat vectors (glue).
  - Launch C: batchnorm + ELU as a per-channel affine in transposed
    layout (bf16 in, f32 out).

All floating-point math runs on device; the host only shards, sorts,
expands by indexing, converts dtypes, and adds a handful of partial
scalars.
"""
import os

import numpy as np
import ml_dtypes

import concourse.bacc as bacc
import concourse.mybir as mybir
import concourse.tile as tile
from concourse import bass_utils
from concourse.vector_clock import ScopedClock

F32 = mybir.dt.float32
BF16 = mybir.dt.bfloat16
FP8 = mybir.dt.float8e4
NPBF = ml_dtypes.bfloat16
NPF8 = mybir.dt.np(FP8)
NEG_SLOPE = 0.2
BN_EPS = 1e-5
NCORES = 8
P = 128
R_HE = 0.5  # fraction of chunks streamed as pre-projected he (256-wide)

LAST_RESULTS = []  # BassKernelResults of the last kernel() call (A, B, C)


def _patch_tile_drain():
    """This walrus build rejects multiple sem waits on the Tile tail Drain
    ("Too many sync wait commands"); move each wait onto its own NOP."""
    if getattr(tile.TileContext, "_gat_drain_patched", False):
        return

    def _drain_and_barrier(self, tick_clock, wait_clock):
        nc = self.nc
        drain_inst = nc.sync.drain()
        wait_clock.add_sem_waits(
            drain_inst.ins, ScopedClock({None: tick_clock.global_clock})
        )
        si = drain_inst.ins.sync_info
        if si is not None and si.on_wait:
            waits = list(si.on_wait)
            drain_inst.ins.sync_info = mybir.SyncInfo(
                on_wait=[], on_update=list(si.on_update)
            )
            for w in waits:
                n = nc.sync.nop(nofuse=True, hint="drain_wait")
                n.ins.sync_info = mybir.SyncInfo(on_wait=[w], on_update=[])
        nc.all_engine_barrier()
        popped = nc._tile_sem_poison_stack.pop()
        assert popped is self._sem_poison
        nc.clear_and_free_semaphores(list(self.sems.allocated().values()))
        nc.all_engine_barrier()

    tile.TileContext._drain_and_barrier = _drain_and_barrier
    tile.TileContext._gat_drain_patched = True


def _run(nc, in_maps, out_names):
    if os.environ.get("GAT_SIM"):
        from concourse.bass_interp import CoreSim

        results = []
        for m in in_maps:
            sim = CoreSim(nc, trace=False, require_finite=False,
                          require_nnan=False)
            for k, v in m.items():
                sim.tensor(k)[:] = v
            sim.simulate()
            results.append({k: np.array(sim.tensor(k)[:]) for k in out_names})

        class R:
            pass

        r = R()
        r.results = results
        r.exec_time_ns = None
        return r
    return bass_utils.run_bass_kernel_spmd(
        nc, in_maps, core_ids=list(range(NCORES)))


def kernel(x, edge_index, edge_attr, W, W_edge, att_src, att_dst, att_edge,
           bias, gamma, beta):
    _patch_tile_drain()
    global LAST_RESULTS
    LAST_RESULTS = []

    x = np.asarray(x, dtype=np.float32)
    edge_index = np.asarray(edge_index)
    edge_attr = np.asarray(edge_attr, dtype=np.float32)
    W = np.asarray(W, dtype=np.float32)
    W_edge = np.asarray(W_edge, dtype=np.float32)
    att_src = np.asarray(att_src, dtype=np.float32)
    att_dst = np.asarray(att_dst, dtype=np.float32)
    att_edge = np.asarray(att_edge, dtype=np.float32)
    gamma = np.asarray(gamma, dtype=np.float32)
    beta = np.asarray(beta, dtype=np.float32)

    N, IN = x.shape
    H, C = att_src.shape
    HC = H * C
    MBW = HC + H  # message row width: HC channels + H denominator slots
    E = edge_index.shape[1]
    assert IN == P and N % NCORES == 0
    SH = N // NCORES
    T = (SH + P - 1) // P
    TF = SH // P          # full tiles
    LROWS = SH - TF * P   # rows in last (partial) tile
    src_all = edge_index[0].astype(np.int64)
    dst_all = edge_index[1].astype(np.int64)
    ea_all = edge_attr[:, 0].astype(np.float32)

    xT = np.ascontiguousarray(x.T)
    xT_bf = xT.astype(NPBF)
    asrc_rep = np.tile(att_src.reshape(1, HC), (P, 1)).astype(np.float32)
    adst_rep = np.tile(att_dst.reshape(1, HC), (P, 1)).astype(np.float32)
    ones_col = np.ones((P, 1), dtype=np.float32)
    ones_bf = np.ones((P, 1), dtype=NPBF)

    # [c,h]-major channel order: innermost (stride-1) per-head dim for the
    # exp-weight broadcast in the message mul.
    old_of_new = (np.arange(H)[None, :] * C
                  + np.arange(C)[:, None]).reshape(-1)  # new j -> old h*C+c
    W_bf = np.ascontiguousarray(W[:, old_of_new]).astype(NPBF)

    # ------------------------------------------------------------------
    # Launch A: h = x@W (bf16), a_src/a_dst tables, we[h], ea partial sums
    # ------------------------------------------------------------------
    EASH = ((E // NCORES) + P - 1) // P * P
    nc = bacc.Bacc("TRN2", target_bir_lowering=False, debug=False)
    d = {}
    d["xT_sh"] = nc.dram_tensor("xT_sh", [P, SH], BF16, kind="ExternalInput")
    d["W"] = nc.dram_tensor("W", [P, HC], F32, kind="ExternalInput")
    d["W_bf"] = nc.dram_tensor("W_bf", [P, HC], BF16, kind="ExternalInput")
    d["asrc_rep"] = nc.dram_tensor("asrc_rep", [P, HC], F32, kind="ExternalInput")
    d["adst_rep"] = nc.dram_tensor("adst_rep", [P, HC], F32, kind="ExternalInput")
    d["wedge"] = nc.dram_tensor("wedge", [1, HC], F32, kind="ExternalInput")
    d["aedge"] = nc.dram_tensor("aedge", [1, HC], F32, kind="ExternalInput")
    d["ea_sh"] = nc.dram_tensor("ea_sh", [P, EASH // P], F32, kind="ExternalInput")
    d["ones_col"] = nc.dram_tensor("ones_col", [P, 1], F32, kind="ExternalInput")
    atab_t = nc.dram_tensor("a_tab", [SH, 2 * H], F32, kind="ExternalOutput")
    h_t = nc.dram_tensor("h_out", [SH, HC], BF16, kind="ExternalOutput")
    we_t = nc.dram_tensor("we_out", [1, H], F32, kind="ExternalOutput")
    eas_t = nc.dram_tensor("ea_sum", [1, 1], F32, kind="ExternalOutput")

    RW = 2 * H + HC  # combined rhs width: [Ws|Wd|W]
    with tile.TileContext(nc) as tc:
        with tc.tile_pool(name="sbuf", bufs=2) as pool, \
             tc.tile_pool(name="hsb", bufs=1) as hpool, \
             tc.tile_pool(name="psum", bufs=4, space="PSUM") as pp:
            w_sb = pool.tile([P, HC], F32, tag="w")
            nc.sync.dma_start(out=w_sb[:], in_=d["W"].ap())
            ar_sb = pool.tile([P, HC], F32, tag="ar")
            nc.sync.dma_start(out=ar_sb[:], in_=d["asrc_rep"].ap())
            ad_sb = pool.tile([P, HC], F32, tag="ad")
            nc.sync.dma_start(out=ad_sb[:], in_=d["adst_rep"].ap())
            on_sb = pool.tile([P, 1], F32, tag="ones")
            nc.sync.dma_start(out=on_sb[:], in_=d["ones_col"].ap())
            # combined projection rhs: [Ws|Wd|W_bf]
            rhs_all = pool.tile([P, RW], BF16, tag="rhs_all")
            nc.sync.dma_start(out=rhs_all[:, 2 * H:RW], in_=d["W_bf"].ap())
            tmp = pool.tile([P, HC], F32, tag="tmp")
            wswd = pool.tile([P, 2 * H], F32, tag="wswd")
            nc.vector.tensor_mul(tmp[:], w_sb[:], ar_sb[:])
            for h in range(H):
                nc.vector.reduce_sum(wswd[:, h:h + 1], tmp[:, h * C:(h + 1) * C],
                                     axis=mybir.AxisListType.X)
            nc.vector.tensor_mul(tmp[:], w_sb[:], ad_sb[:])
            for h in range(H):
                nc.vector.reduce_sum(wswd[:, H + h:H + h + 1],
                                     tmp[:, h * C:(h + 1) * C],
                                     axis=mybir.AxisListType.X)
            nc.vector.tensor_copy(rhs_all[:, 0:2 * H], wswd[:])
            we_row = pool.tile([1, HC], F32, tag="we_row")
            nc.sync.dma_start(out=we_row[:], in_=d["wedge"].ap())
            ae_row = pool.tile([1, HC], F32, tag="ae_row")
            nc.sync.dma_start(out=ae_row[:], in_=d["aedge"].ap())
            nc.vector.tensor_mul(we_row[:], we_row[:], ae_row[:])
            we_sb = pool.tile([1, H], F32, tag="we_sb")
            for h in range(H):
                nc.vector.reduce_sum(we_sb[:, h:h + 1],
                                     we_row[:, h * C:(h + 1) * C],
                                     axis=mybir.AxisListType.X)
            nc.sync.dma_start(out=we_t.ap(), in_=we_sb[:])
            ea_sb = pool.tile([P, EASH // P], F32, tag="ea")
            nc.sync.dma_start(out=ea_sb[:], in_=d["ea_sh"].ap())
            red = pool.tile([P, 1], F32, tag="red")
            nc.vector.reduce_sum(red[:], ea_sb[:], axis=mybir.AxisListType.X)
            ps1 = pp.tile([1, 1], F32, tag="ps1")
            nc.tensor.matmul(ps1[:], lhsT=on_sb[:], rhs=red[:], start=True,
                             stop=True)
            sc = pool.tile([1, 1], F32, tag="sc")
            nc.vector.tensor_copy(sc[:], ps1[:])
            nc.sync.dma_start(out=eas_t.ap(), in_=sc[:])

            xsh = pool.tile([P, SH], BF16, tag="xsh")
            nc.sync.dma_start(out=xsh[:], in_=d["xT_sh"].ap())
            atab_sb = pool.tile([P, T * 2 * H], F32, tag="atab")
            h_sb = hpool.tile([P, T * HC], BF16, tag="hsb")
            for t in range(T):
                rows = min(P, SH - t * P)
                ps = pp.tile([P, RW], F32, tag="ps")
                nc.tensor.matmul(ps[:rows, :], lhsT=xsh[:, t * P:t * P + rows],
                                 rhs=rhs_all[:], start=True, stop=True)
                nc.vector.tensor_copy(atab_sb[:rows, t * 2 * H:(t + 1) * 2 * H],
                                      ps[:rows, 0:2 * H])
                if t % 2 == 0:
                    nc.scalar.activation(h_sb[:rows, t * HC:(t + 1) * HC],
                                         ps[:rows, 2 * H:RW],
                                         mybir.ActivationFunctionType.Copy)
                else:
                    nc.vector.tensor_copy(h_sb[:rows, t * HC:(t + 1) * HC],
                                          ps[:rows, 2 * H:RW])
            nc.sync.dma_start(
                out=atab_t.ap()[0:TF * P, :].rearrange("(t p) h -> p t h", p=P),
                in_=atab_sb[:, :TF * 2 * H].rearrange("p (t h) -> p t h",
                                                      h=2 * H))
            if LROWS:
                nc.sync.dma_start(
                    out=atab_t.ap()[TF * P:SH, :],
                    in_=atab_sb[:LROWS, TF * 2 * H:T * 2 * H])
            nc.sync.dma_start(
                out=h_t.ap()[0:TF * P, :].rearrange("(t p) h -> p t h", p=P),
                in_=h_sb[:, :TF * HC].rearrange("p (t h) -> p t h", h=HC))
            if LROWS:
                nc.sync.dma_start(
                    out=h_t.ap()[TF * P:SH, :],
                    in_=h_sb[:LROWS, TF * HC:T * HC])
    nc.compile()

    in_maps = []
    for c in range(NCORES):
        ea_sl = np.zeros(EASH, dtype=np.float32)
        lo, hi = c * (E // NCORES), (c + 1) * (E // NCORES)
        if c == NCORES - 1:
            hi = E
        seg = ea_all[lo:hi]
        ea_sl[:seg.shape[0]] = seg
        in_maps.append({
            "xT_sh": np.ascontiguousarray(xT_bf[:, c * SH:(c + 1) * SH]),
            "W": W, "W_bf": W_bf, "asrc_rep": asrc_rep, "adst_rep": adst_rep,
            "wedge": W_edge.reshape(1, HC).astype(np.float32),
            "aedge": att_edge.reshape(1, HC).astype(np.float32),
            "ea_sh": np.ascontiguousarray(ea_sl.reshape(EASH // P, P).T),
            "ones_col": ones_col,
        })
    resA = _run(nc, in_maps, ["a_tab", "h_out", "we_out", "ea_sum"])
    LAST_RESULTS.append(resA)

    a_tab = np.concatenate([r["a_tab"] for r in resA.results], axis=0)
    h_all = np.concatenate([np.asarray(r["h_out"]) for r in resA.results],
                           axis=0)  # [N, HC] bf16, permuted channel order
    we = resA.results[0]["we_out"][0].astype(np.float32)
    ea_mean = float(sum(float(r["ea_sum"][0, 0]) for r in resA.results)) / E

    # ------------------------------------------------------------------
    # Host: edges (+self-loops) -> per-core per-dst-tile chunk slots
    # ------------------------------------------------------------------
    loops = np.arange(N, dtype=np.int64)
    src_x = np.concatenate([src_all, loops])
    dst_x = np.concatenate([dst_all, loops])
    ea_x = np.concatenate([ea_all, np.full(N, ea_mean, dtype=np.float32)])

    per_core = []
    for c in range(NCORES):
        m = (dst_x >= c * SH) & (dst_x < (c + 1) * SH)
        s, dd, ee = src_x[m], dst_x[m] - c * SH, ea_x[m]
        order = np.argsort(dd, kind="stable")
        s, dd, ee = s[order], dd[order], ee[order]
        tb = dd // P
        bounds = np.searchsorted(tb, np.arange(T + 1))
        per_core.append((s, dd, ee, bounds))

    nch = [max(int(per_core[c][3][t + 1] - per_core[c][3][t] + P - 1) // P
               for c in range(NCORES)) for t in range(T)]
    hks = [min(t_n, int(round(R_HE * t_n))) for t_n in nch]
    xks = [t_n - h_k for t_n, h_k in zip(nch, hks)]
    NCH = sum(nch)
    NCHE = sum(hks)
    NCHX = sum(xks)
    TOTMAX = max(nch)
    offs = np.concatenate([[0], np.cumsum(nch)]).astype(np.int64)
    offse = np.concatenate([[0], np.cumsum(hks)]).astype(np.int64)
    offsx = np.concatenate([[0], np.cumsum(xks)]).astype(np.int64)

    FLD = 2 * H + 1  # packed per-edge fields: asrc(8) adst(8) ea(1)
    core_inputs = []
    for c in range(NCORES):
        s, dd, ee, bounds = per_core[c]
        gsrc = np.zeros(NCH * P, dtype=np.int64)
        pad = np.ones(NCH * P, dtype=bool)
        drel_all = np.zeros(NCH * P, dtype=np.int64)
        for t in range(T):
            lo, hi = int(bounds[t]), int(bounds[t + 1])
            n = hi - lo
            base = int(offs[t]) * P
            gsrc[base:base + n] = s[lo:hi]
            pad[base:base + n] = False
            drel_all[base:base + n] = dd[lo:hi] - t * P
        # he stream: [128 slot, NCHE*256] bf16, chunk k cols = h[src] rows
        he = np.zeros((P, NCHE * HC), dtype=NPBF)
        # xe stream: [128 xdim, NCHX*128] bf16, col = xT[:, src]
        xe = np.zeros((P, NCHX * P), dtype=NPBF)
        for t in range(T):
            tot, hk = nch[t], hks[t]
            b0 = int(offs[t]) * P
            # he chunks: slots [b0, b0+hk*128)
            if hk:
                sl = slice(b0, b0 + hk * P)
                blk = h_all[gsrc[sl]]              # [hk*128, 256]
                blk[pad[sl]] = NPBF(0)
                he[:, int(offse[t]) * HC:(int(offse[t]) + hk) * HC] = (
                    blk.reshape(hk, P, HC).transpose(1, 0, 2)
                    .reshape(P, hk * HC))
            # xe chunks: slots [b0+hk*128, b0+tot*128)
            xk = tot - hk
            if xk:
                sl = slice(b0 + hk * P, b0 + tot * P)
                xblk = xT_bf[:, gsrc[sl]]          # [128, xk*128]
                xblk[:, pad[sl]] = NPBF(0)
                xe[:, int(offsx[t]) * P:(int(offsx[t]) + xk) * P] = xblk
        # one-hot dst-selection matrices (fp8), precomputed host-side
        soh = np.zeros((NCH, P, P), dtype=NPF8)
        real = ~pad
        slot = np.nonzero(real)[0]
        soh[slot // P, slot % P, drel_all[real]] = NPF8(1)
        soh = np.ascontiguousarray(
            soh.transpose(1, 0, 2).reshape(P, NCH * P))
        # packed per-edge scalars, field-major per tile
        pk = np.zeros((P, NCH * FLD), dtype=np.float32)
        for t in range(T):
            lo, hi = int(bounds[t]), int(bounds[t + 1])
            n = hi - lo
            tot = nch[t]
            sl = slice(lo, hi)

            def expand(vals, w):
                buf = np.zeros((tot * P, w), dtype=np.float32)
                buf[:n] = vals.reshape(n, w)
                return (buf.reshape(tot, P, w).transpose(1, 0, 2)
                        .reshape(P, tot * w))

            b0 = int(offs[t]) * FLD
            pk[:, b0:b0 + tot * H] = expand(a_tab[s[sl], 0:H], H)
            pk[:, b0 + tot * H:b0 + 2 * tot * H] = expand(
                a_tab[c * SH + dd[sl], H:2 * H], H)
            pk[:, b0 + 2 * tot * H:b0 + FLD * tot] = expand(ee[sl], 1)
        core_inputs.append(dict(he=he, xe=xe, soh=soh, pk=pk.astype(NPBF)))

    we_tiled = np.ascontiguousarray(
        np.tile(we.reshape(1, 1, H), (P, TOTMAX, 1))
        .reshape(P, TOTMAX * H)).astype(NPBF)

    # ------------------------------------------------------------------
    # Launch B
    # ------------------------------------------------------------------
    nc = bacc.Bacc("TRN2", target_bir_lowering=False, debug=False)
    he_t = nc.dram_tensor("he", [P, max(NCHE, 1) * HC], BF16,
                          kind="ExternalInput")
    xe_t = nc.dram_tensor("xe", [P, max(NCHX, 1) * P], BF16,
                          kind="ExternalInput")
    soh_t = nc.dram_tensor("soh", [P, NCH * P], FP8, kind="ExternalInput")
    pk_t = nc.dram_tensor("pk", [P, NCH * FLD], BF16, kind="ExternalInput")
    W_t = nc.dram_tensor("W", [P, HC], BF16, kind="ExternalInput")
    onesb_t = nc.dram_tensor("ones_bf", [P, 1], BF16, kind="ExternalInput")
    wet_t = nc.dram_tensor("we_tiled", [P, TOTMAX * H], BF16,
                           kind="ExternalInput")
    opre_t = nc.dram_tensor("out_pre", [SH, HC], BF16, kind="ExternalOutput")
    stats_t = nc.dram_tensor("stats", [1, 2 * HC], F32, kind="ExternalOutput")

    HKMAX = max(max(hks), 1)
    XKMAX = max(max(xks), 1)
    with tile.TileContext(nc) as tc:
        with tc.tile_pool(name="const", bufs=1) as cpool:
            w_sb = cpool.tile([P, HC], BF16, tag="w")
            nc.sync.dma_start(out=w_sb[:], in_=W_t.ap())
            on_sb = cpool.tile([P, 1], BF16, tag="ones")
            nc.sync.dma_start(out=on_sb[:], in_=onesb_t.ap())
            wet_sb = cpool.tile([P, TOTMAX * H], BF16, tag="wet")
            nc.sync.dma_start(out=wet_sb[:], in_=wet_t.ap())

            with tc.tile_pool(name="he", bufs=3) as hepool, \
                 tc.tile_pool(name="xe", bufs=3) as xpool, \
                 tc.tile_pool(name="pk", bufs=3) as kpool, \
                 tc.tile_pool(name="mb", bufs=2) as mpool, \
                 tc.tile_pool(name="s", bufs=3) as spool, \
                 tc.tile_pool(name="fin", bufs=3) as fpool, \
                 tc.tile_pool(name="hp", bufs=3, space="PSUM") as hpp, \
                 tc.tile_pool(name="acc", bufs=2, space="PSUM") as apool, \
                 tc.tile_pool(name="stp", bufs=1, space="PSUM") as stpool:
                stats_ps = stpool.tile([1, 2 * HC], F32, tag="stats")
                for t in range(T):
                    rows = min(P, SH - t * P)
                    tot, hk, xk = nch[t], hks[t], xks[t]
                    off = int(offs[t])
                    offe = int(offse[t])
                    offx = int(offsx[t])
                    he_sb = None
                    if hk:
                        he_sb = hepool.tile([P, HKMAX * HC], BF16, tag="he")
                        nc.sync.dma_start(
                            out=he_sb[:, :hk * HC],
                            in_=he_t.ap()[:, offe * HC:(offe + hk) * HC])
                    xe_sb = None
                    if xk:
                        xe_sb = xpool.tile([P, XKMAX * P], BF16, tag="xe")
                        nc.sync.dma_start(
                            out=xe_sb[:, :xk * P],
                            in_=xe_t.ap()[:, offx * P:(offx + xk) * P])
                    s_sb = spool.tile([P, TOTMAX * P], FP8, tag="S")
                    nc.sync.dma_start(out=s_sb[:, :tot * P],
                                      in_=soh_t.ap()[:, off * P:(off + tot) * P])
                    pk_sb = kpool.tile([P, TOTMAX * FLD], BF16, tag="pk")
                    nc.sync.dma_start(
                        out=pk_sb[:, :tot * FLD],
                        in_=pk_t.ap()[:, off * FLD:(off + tot) * FLD])
                    a1 = pk_sb[:, 0:tot * H]
                    a2 = pk_sb[:, tot * H:2 * tot * H]
                    eav = pk_sb[:, 2 * tot * H:2 * tot * H + tot]
                    # alpha = asrc + adst + ea*we ; w = exp(leaky_relu(alpha))
                    nc.vector.tensor_add(a1, a1, a2)
                    nc.vector.tensor_mul(
                        a2.rearrange("p (k h) -> p k h", h=H),
                        eav.to_broadcast([P, tot, H]),
                        wet_sb[:, :tot * H].rearrange("p (k h) -> p k h", h=H))
                    nc.vector.tensor_add(a1, a1, a2)
                    nc.scalar.activation(a2, a1,
                                         mybir.ActivationFunctionType.Relu,
                                         scale=-float(1.0 - NEG_SLOPE))
                    nc.vector.tensor_add(a1, a1, a2)
                    mb = mpool.tile([P, TOTMAX * MBW], BF16, tag="mb")
                    mbv = mb[:, :tot * MBW].rearrange("p (k e) -> p k e", e=MBW)
                    nc.scalar.activation(mbv[:, :, HC:MBW],
                                         a1.rearrange("p (k h) -> p k h", h=H),
                                         mybir.ActivationFunctionType.Exp)
                    acc = apool.tile([P, MBW], F32, tag="acc")

                    def ekb_for(k0, run):
                        return (mb[:, k0 * MBW:(k0 + run) * MBW]
                                .rearrange("p (k e) -> p k e", e=MBW)
                                [:, :, HC:MBW]
                                .rearrange("p k (o h) -> p k o h", o=1)
                                .to_broadcast([P, run, C, H]))

                    def mb_msg(k0, run):
                        return (mb[:, k0 * MBW:(k0 + run) * MBW]
                                .rearrange("p (k e) -> p k e", e=MBW)
                                [:, :, 0:HC]
                                .rearrange("p k (c h) -> p k c h", h=H))

                    def scat(k):
                        nc.tensor.matmul(
                            acc[:], lhsT=s_sb[:, k * P:(k + 1) * P],
                            rhs=mb[:, k * MBW:(k + 1) * MBW],
                            start=(k == 0), stop=(k == tot - 1))

                    # he chunks: k in [0, hk) — mul straight from SBUF
                    for k0 in range(0, hk, 4):
                        run = min(4, hk - k0)
                        nc.vector.tensor_mul(
                            mb_msg(k0, run),
                            he_sb[:, k0 * HC:(k0 + run) * HC].rearrange(
                                "p (k c h) -> p k c h", c=C, h=H),
                            ekb_for(k0, run))
                        for j in range(run):
                            scat(k0 + j)
                    # xe chunks: k in [hk, tot) — project on PE, then mul
                    for k0 in range(hk, tot, 2):
                        run = min(2, tot - k0)
                        hps = hpp.tile([P, 2 * HC], F32, tag="hps")
                        for j in range(run):
                            nc.tensor.matmul(
                                hps[:, j * HC:(j + 1) * HC],
                                lhsT=xe_sb[:, (k0 + j - hk) * P:
                                           (k0 + j - hk + 1) * P],
                                rhs=w_sb[:], start=True, stop=True)
                        nc.vector.tensor_mul(
                            mb_msg(k0, run),
                            hps[:, :run * HC].rearrange(
                                "p (k c h) -> p k c h", c=C, h=H),
                            ekb_for(k0, run))
                        for j in range(run):
                            scat(k0 + j)
                    # finalize tile: normalize + stats
                    den = fpool.tile([P, H], F32, tag="den")
                    nc.vector.tensor_copy(den[:rows], acc[:rows, HC:MBW])
                    rec = fpool.tile([P, H], F32, tag="rec")
                    nc.vector.reciprocal(rec[:rows], den[:rows])
                    opsq = fpool.tile([P, 2 * HC], BF16, tag="opsq")
                    nc.vector.tensor_mul(
                        opsq[:rows, :HC].rearrange("p (c h) -> p c h", h=H),
                        acc[:rows, :HC].rearrange("p (c h) -> p c h", h=H),
                        rec[0:rows, :].rearrange("p (o h) -> p o h",
                                                 o=1).to_broadcast(
                            [rows, C, H]))
                    nc.scalar.activation(opsq[:rows, HC:], opsq[:rows, :HC],
                                         mybir.ActivationFunctionType.Square)
                    nc.tensor.matmul(stats_ps[:, :], lhsT=on_sb[:rows, :],
                                     rhs=opsq[:rows, :], start=(t == 0),
                                     stop=(t == T - 1))
                    nc.sync.dma_start(out=opre_t.ap()[t * P:t * P + rows, :],
                                      in_=opsq[:rows, :HC])
                st_sb = fpool.tile([1, 2 * HC], F32, tag="stsb")
                nc.vector.tensor_copy(st_sb[:], stats_ps[:])
                nc.sync.dma_start(out=stats_t.ap(), in_=st_sb[:])
    nc.compile()

    in_maps = []
    for c in range(NCORES):
        ci = core_inputs[c]
        in_maps.append({
            "he": ci["he"], "xe": ci["xe"], "soh": ci["soh"], "pk": ci["pk"],
            "W": W_bf, "ones_bf": ones_bf, "we_tiled": we_tiled,
        })
    resB = _run(nc, in_maps, ["out_pre", "stats"])
    LAST_RESULTS.append(resB)

    out_pre = np.concatenate([np.asarray(r["out_pre"])
                              for r in resB.results], axis=0)
    stats = np.stack([np.asarray(r["stats"][0], dtype=np.float64)
                      for r in resB.results]).sum(axis=0).astype(np.float32)
    sums_col = np.ascontiguousarray(
        np.stack([stats[:HC], stats[HC:]], axis=1))  # [HC, 2]

    # ------------------------------------------------------------------
    # Launch C: batchnorm + ELU (transposed layout)
    # ------------------------------------------------------------------
    opT = np.ascontiguousarray(out_pre.reshape(NCORES, SH, HC)
                               .transpose(0, 2, 1))  # [8, HC, SH] bf16
    nc = bacc.Bacc("TRN2", target_bir_lowering=False, debug=False)
    opT_t = nc.dram_tensor("opT", [HC, SH], BF16, kind="ExternalInput")
    sums_t = nc.dram_tensor("sums_col", [HC, 2], F32, kind="ExternalInput")
    gam_t = nc.dram_tensor("gamma_col", [HC, 1], F32, kind="ExternalInput")
    bet_t = nc.dram_tensor("beta_col", [HC, 1], F32, kind="ExternalInput")
    outT_t = nc.dram_tensor("outT", [HC, SH], F32, kind="ExternalOutput")

    CT = HC // P
    with tile.TileContext(nc) as tc:
        with tc.tile_pool(name="sbuf", bufs=2) as pool:
            for ct in range(CT):
                sm = pool.tile([P, 2], F32, tag="sm")
                nc.sync.dma_start(out=sm[:], in_=sums_t.ap()[ct * P:(ct + 1) * P, :])
                gm = pool.tile([P, 1], F32, tag="gm")
                nc.sync.dma_start(out=gm[:], in_=gam_t.ap()[ct * P:(ct + 1) * P, :])
                bt = pool.tile([P, 1], F32, tag="bt")
                nc.sync.dma_start(out=bt[:], in_=bet_t.ap()[ct * P:(ct + 1) * P, :])
                mean = pool.tile([P, 1], F32, tag="mean")
                nc.vector.tensor_scalar_mul(mean[:], sm[:, 0:1], 1.0 / N)
                ex2 = pool.tile([P, 1], F32, tag="ex2")
                nc.vector.tensor_scalar_mul(ex2[:], sm[:, 1:2], 1.0 / N)
                msq = pool.tile([P, 1], F32, tag="msq")
                nc.vector.tensor_mul(msq[:], mean[:], mean[:])
                var = pool.tile([P, 1], F32, tag="var")
                nc.vector.tensor_sub(var[:], ex2[:], msq[:])
                nc.vector.tensor_scalar_add(var[:], var[:], float(BN_EPS))
                sd = pool.tile([P, 1], F32, tag="sd")
                nc.scalar.activation(sd[:], var[:],
                                     mybir.ActivationFunctionType.Sqrt)
                inv = pool.tile([P, 1], F32, tag="inv")
                nc.vector.reciprocal(inv[:], sd[:])
                scl = pool.tile([P, 1], F32, tag="scl")
                nc.vector.tensor_mul(scl[:], inv[:], gm[:])
                sh1 = pool.tile([P, 1], F32, tag="sh1")
                nc.vector.tensor_mul(sh1[:], mean[:], scl[:])
                shf = pool.tile([P, 1], F32, tag="shf")
                nc.vector.tensor_sub(shf[:], bt[:], sh1[:])
                CW = SH // 2
                for cs in range(2):
                    c0 = cs * CW
                    xt_ = pool.tile([P, CW], BF16, tag="xt")
                    nc.sync.dma_start(
                        out=xt_[:],
                        in_=opT_t.ap()[ct * P:(ct + 1) * P, c0:c0 + CW])
                    y = pool.tile([P, CW], F32, tag="y")
                    nc.scalar.activation(y[:], xt_[:],
                                         mybir.ActivationFunctionType.Identity,
                                         bias=shf[:], scale=scl[:])
                    r = pool.tile([P, CW], F32, tag="r")
                    nc.vector.tensor_scalar_max(r[:], y[:], 0.0)
                    yneg = pool.tile([P, CW], F32, tag="yneg")
                    nc.vector.tensor_sub(yneg[:], y[:], r[:])
                    e = pool.tile([P, CW], F32, tag="e")
                    nc.scalar.activation(e[:], yneg[:],
                                         mybir.ActivationFunctionType.Exp)
                    nc.vector.tensor_scalar_add(r[:], r[:], -1.0)
                    nc.vector.tensor_add(r[:], r[:], e[:])
                    nc.sync.dma_start(
                        out=outT_t.ap()[ct * P:(ct + 1) * P, c0:c0 + CW],
                        in_=r[:])
    nc.compile()

    in_maps = [{
        "opT": np.ascontiguousarray(opT[c]),
        "sums_col": sums_col,
        "gamma_col": gamma[old_of_new].reshape(HC, 1),
        "beta_col": beta[old_of_new].reshape(HC, 1),
    } for c in range(NCORES)]
    resC = _run(nc, in_maps, ["outT"])
    LAST_RESULTS.append(resC)

    outp = np.concatenate(
        [np.asarray(r["outT"]).T for r in resC.results], axis=0)  # [N, HC]
    out = np.empty_like(outp)
    out[:, old_of_new] = outp  # undo the [c,h] channel permutation
    return np.ascontiguousarray(out.astype(np.float32))


# revision 11
# speedup vs baseline: 1.1854x; 1.1058x over previous
"""Multi-head GAT layer (GATConv + BatchNorm + ELU) on 8 trn2 NeuronCores.

Dst-sharded graph parallelism, v2 (mixed he/xe streaming):
  - Launch A: per-node h = x@W (bf16, [c-major,h-minor] channel order),
    a_src/a_dst tables, per-head edge coefficient we[h], partial sums of
    edge_attr (for the self-loop fill value).
  - Host: adds self-loops, buckets edges per 128-dst tile, expands per-edge
    streams BY INDEXING ONLY: a fraction R_HE of each tile's 128-edge
    chunks stream the PRE-PROJECTED source features he = h[src] (bf16,
    256 wide -> no PE projection matmul needed), the rest stream raw
    source features xe = xT[:, src] (bf16, 128 wide -> cheaper DMA but
    a PE projection per chunk).  One-hot dst-scatter matrices are sent
    in fp8 (exact for 0/1, half the DMA, 4x faster weight load).
  - Launch B: per dst tile, per 128-edge chunk:
      w     = exp(leaky_relu(asrc+adst+ea*we))        (vec/scalar, per tile)
      mb    = [he*w | w]  or  [(xe@W)*w | w]          (one fused DVE mul)
      acc  += onehot_fp8^T @ mb                       (PE scatter-add, PSUM)
    then normalizes by the per-dst denominator, emits bf16 out_pre rows and
    per-channel sum/sumsq stats via a ones-matmul.
  - Host: sums the 8 partial stat vectors (glue).
  - Launch C: batchnorm + ELU as a per-channel affine in transposed
    layout (bf16 in, f32 out).

All floating-point math runs on device; the host only shards, sorts,
expands by indexing, converts dtypes, and adds a handful of partial
scalars.
"""
import os

import numpy as np
import ml_dtypes

import concourse.bacc as bacc
import concourse.mybir as mybir
import concourse.tile as tile
from concourse import bass_utils
from concourse.vector_clock import ScopedClock

F32 = mybir.dt.float32
BF16 = mybir.dt.bfloat16
FP8 = mybir.dt.float8e4
NPBF = ml_dtypes.bfloat16
NPF8 = mybir.dt.np(FP8)
NEG_SLOPE = 0.2
BN_EPS = 1e-5
NCORES = 8
P = 128
R_HE = 0.6  # fraction of chunks streamed as pre-projected he (256-wide)

LAST_RESULTS = []  # BassKernelResults of the last kernel() call (A, B, C)


def _patch_tile_drain():
    """This walrus build rejects multiple sem waits on the Tile tail Drain
    ("Too many sync wait commands"); move each wait onto its own NOP."""
    if getattr(tile.TileContext, "_gat_drain_patched", False):
        return

    def _drain_and_barrier(self, tick_clock, wait_clock):
        nc = self.nc
        drain_inst = nc.sync.drain()
        wait_clock.add_sem_waits(
            drain_inst.ins, ScopedClock({None: tick_clock.global_clock})
        )
        si = drain_inst.ins.sync_info
        if si is not None and si.on_wait:
            waits = list(si.on_wait)
            drain_inst.ins.sync_info = mybir.SyncInfo(
                on_wait=[], on_update=list(si.on_update)
            )
            for w in waits:
                n = nc.sync.nop(nofuse=True, hint="drain_wait")
                n.ins.sync_info = mybir.SyncInfo(on_wait=[w], on_update=[])
        nc.all_engine_barrier()
        popped = nc._tile_sem_poison_stack.pop()
        assert popped is self._sem_poison
        nc.clear_and_free_semaphores(list(self.sems.allocated().values()))
        nc.all_engine_barrier()

    tile.TileContext._drain_and_barrier = _drain_and_barrier
    tile.TileContext._gat_drain_patched = True


def _run(nc, in_maps, out_names):
    if os.environ.get("GAT_SIM"):
        from concourse.bass_interp import CoreSim

        results = []
        for m in in_maps:
            sim = CoreSim(nc, trace=False, require_finite=False,
                          require_nnan=False)
            for k, v in m.items():
                sim.tensor(k)[:] = v
            sim.simulate()
            results.append({k: np.array(sim.tensor(k)[:]) for k in out_names})

        class R:
            pass

        r = R()
        r.results = results
        r.exec_time_ns = None
        return r
    return bass_utils.run_bass_kernel_spmd(
        nc, in_maps, core_ids=list(range(NCORES)))


def kernel(x, edge_index, edge_attr, W, W_edge, att_src, att_dst, att_edge,
           bias, gamma, beta):
    _patch_tile_drain()
    global LAST_RESULTS
    LAST_RESULTS = []

    x = np.asarray(x, dtype=np.float32)
    edge_index = np.asarray(edge_index)
    edge_attr = np.asarray(edge_attr, dtype=np.float32)
    W = np.asarray(W, dtype=np.float32)
    W_edge = np.asarray(W_edge, dtype=np.float32)
    att_src = np.asarray(att_src, dtype=np.float32)
    att_dst = np.asarray(att_dst, dtype=np.float32)
    att_edge = np.asarray(att_edge, dtype=np.float32)
    gamma = np.asarray(gamma, dtype=np.float32)
    beta = np.asarray(beta, dtype=np.float32)

    N, IN = x.shape
    H, C = att_src.shape
    HC = H * C
    MBW = HC + H  # message row width: HC channels + H denominator slots
    E = edge_index.shape[1]
    assert IN == P and N % NCORES == 0
    SH = N // NCORES
    T = (SH + P - 1) // P
    TF = SH // P          # full tiles
    LROWS = SH - TF * P   # rows in last (partial) tile
    src_all = edge_index[0].astype(np.int64)
    dst_all = edge_index[1].astype(np.int64)
    ea_all = edge_attr[:, 0].astype(np.float32)

    xT = np.ascontiguousarray(x.T)
    xT_bf = xT.astype(NPBF)
    asrc_rep = np.tile(att_src.reshape(1, HC), (P, 1)).astype(np.float32)
    adst_rep = np.tile(att_dst.reshape(1, HC), (P, 1)).astype(np.float32)
    ones_col = np.ones((P, 1), dtype=np.float32)
    ones_bf = np.ones((P, 1), dtype=NPBF)

    # [c,h]-major channel order: innermost (stride-1) per-head dim for the
    # exp-weight broadcast in the message mul.
    old_of_new = (np.arange(H)[None, :] * C
                  + np.arange(C)[:, None]).reshape(-1)  # new j -> old h*C+c
    W_bf = np.ascontiguousarray(W[:, old_of_new]).astype(NPBF)

    # ------------------------------------------------------------------
    # Launch A: h = x@W (bf16), a_src/a_dst tables, we[h], ea partial sums
    # ------------------------------------------------------------------
    EASH = ((E // NCORES) + P - 1) // P * P
    nc = bacc.Bacc("TRN2", target_bir_lowering=False, debug=False)
    d = {}
    d["xT_sh"] = nc.dram_tensor("xT_sh", [P, SH], BF16, kind="ExternalInput")
    d["W"] = nc.dram_tensor("W", [P, HC], F32, kind="ExternalInput")
    d["W_bf"] = nc.dram_tensor("W_bf", [P, HC], BF16, kind="ExternalInput")
    d["asrc_rep"] = nc.dram_tensor("asrc_rep", [P, HC], F32, kind="ExternalInput")
    d["adst_rep"] = nc.dram_tensor("adst_rep", [P, HC], F32, kind="ExternalInput")
    d["wedge"] = nc.dram_tensor("wedge", [1, HC], F32, kind="ExternalInput")
    d["aedge"] = nc.dram_tensor("aedge", [1, HC], F32, kind="ExternalInput")
    d["ea_sh"] = nc.dram_tensor("ea_sh", [P, EASH // P], BF16, kind="ExternalInput")
    d["ones_col"] = nc.dram_tensor("ones_col", [P, 1], F32, kind="ExternalInput")
    atab_t = nc.dram_tensor("a_tab", [SH, 2 * H], F32, kind="ExternalOutput")
    h_t = nc.dram_tensor("h_out", [SH, HC], BF16, kind="ExternalOutput")
    we_t = nc.dram_tensor("we_out", [1, H], F32, kind="ExternalOutput")
    eas_t = nc.dram_tensor("ea_sum", [1, 1], F32, kind="ExternalOutput")

    RW = 2 * H + HC  # combined rhs width: [Ws|Wd|W]
    with tile.TileContext(nc) as tc:
        with tc.tile_pool(name="sbuf", bufs=2) as pool, \
             tc.tile_pool(name="hsb", bufs=1) as hpool, \
             tc.tile_pool(name="psum", bufs=4, space="PSUM") as pp:
            w_sb = pool.tile([P, HC], F32, tag="w")
            nc.sync.dma_start(out=w_sb[:], in_=d["W"].ap())
            ar_sb = pool.tile([P, HC], F32, tag="ar")
            nc.sync.dma_start(out=ar_sb[:], in_=d["asrc_rep"].ap())
            ad_sb = pool.tile([P, HC], F32, tag="ad")
            nc.sync.dma_start(out=ad_sb[:], in_=d["adst_rep"].ap())
            on_sb = pool.tile([P, 1], F32, tag="ones")
            nc.sync.dma_start(out=on_sb[:], in_=d["ones_col"].ap())
            # combined projection rhs: [Ws|Wd|W_bf]
            rhs_all = pool.tile([P, RW], BF16, tag="rhs_all")
            nc.sync.dma_start(out=rhs_all[:, 2 * H:RW], in_=d["W_bf"].ap())
            tmp = pool.tile([P, HC], F32, tag="tmp")
            wswd = pool.tile([P, 2 * H], F32, tag="wswd")
            nc.vector.tensor_mul(tmp[:], w_sb[:], ar_sb[:])
            for h in range(H):
                nc.vector.reduce_sum(wswd[:, h:h + 1], tmp[:, h * C:(h + 1) * C],
                                     axis=mybir.AxisListType.X)
            nc.vector.tensor_mul(tmp[:], w_sb[:], ad_sb[:])
            for h in range(H):
                nc.vector.reduce_sum(wswd[:, H + h:H + h + 1],
                                     tmp[:, h * C:(h + 1) * C],
                                     axis=mybir.AxisListType.X)
            nc.vector.tensor_copy(rhs_all[:, 0:2 * H], wswd[:])
            we_row = pool.tile([1, HC], F32, tag="we_row")
            nc.sync.dma_start(out=we_row[:], in_=d["wedge"].ap())
            ae_row = pool.tile([1, HC], F32, tag="ae_row")
            nc.sync.dma_start(out=ae_row[:], in_=d["aedge"].ap())
            nc.vector.tensor_mul(we_row[:], we_row[:], ae_row[:])
            we_sb = pool.tile([1, H], F32, tag="we_sb")
            for h in range(H):
                nc.vector.reduce_sum(we_sb[:, h:h + 1],
                                     we_row[:, h * C:(h + 1) * C],
                                     axis=mybir.AxisListType.X)
            nc.sync.dma_start(out=we_t.ap(), in_=we_sb[:])
            ea_sb = pool.tile([P, EASH // P], BF16, tag="ea")
            nc.sync.dma_start(out=ea_sb[:], in_=d["ea_sh"].ap())
            red = pool.tile([P, 1], F32, tag="red")
            nc.vector.reduce_sum(red[:], ea_sb[:], axis=mybir.AxisListType.X)
            ps1 = pp.tile([1, 1], F32, tag="ps1")
            nc.tensor.matmul(ps1[:], lhsT=on_sb[:], rhs=red[:], start=True,
                             stop=True)
            sc = pool.tile([1, 1], F32, tag="sc")
            nc.vector.tensor_copy(sc[:], ps1[:])
            nc.sync.dma_start(out=eas_t.ap(), in_=sc[:])

            xsh = pool.tile([P, SH], BF16, tag="xsh")
            XQ = 8  # tiles per input-DMA / output-DMA group
            for g in range(0, T, XQ):
                ge = min(g + XQ, T)
                nc.sync.dma_start(out=xsh[:, g * P:min(ge * P, SH)],
                                  in_=d["xT_sh"].ap()[:, g * P:min(ge * P, SH)])
            atab_sb = pool.tile([P, T * 2 * H], F32, tag="atab")
            h_sb = hpool.tile([P, T * HC], BF16, tag="hsb")
            for t in range(T):
                rows = min(P, SH - t * P)
                ps = pp.tile([P, RW], F32, tag="ps")
                nc.tensor.matmul(ps[:rows, :], lhsT=xsh[:, t * P:t * P + rows],
                                 rhs=rhs_all[:], start=True, stop=True)
                nc.vector.tensor_copy(atab_sb[:rows, t * 2 * H:(t + 1) * 2 * H],
                                      ps[:rows, 0:2 * H])
                if t % 2 == 0:
                    nc.scalar.activation(h_sb[:rows, t * HC:(t + 1) * HC],
                                         ps[:rows, 2 * H:RW],
                                         mybir.ActivationFunctionType.Copy)
                else:
                    nc.vector.tensor_copy(h_sb[:rows, t * HC:(t + 1) * HC],
                                          ps[:rows, 2 * H:RW])
                # stream h out per completed full-tile group
                if (t + 1) % XQ == 0 or t == T - 1:
                    g0 = (t // XQ) * XQ
                    g1 = min(t + 1, TF)  # full tiles only in this group
                    if g1 > g0:
                        nc.sync.dma_start(
                            out=h_t.ap()[g0 * P:g1 * P, :].rearrange(
                                "(t p) h -> p t h", p=P),
                            in_=h_sb[:, g0 * HC:g1 * HC].rearrange(
                                "p (t h) -> p t h", h=HC))
            nc.sync.dma_start(
                out=atab_t.ap()[0:TF * P, :].rearrange("(t p) h -> p t h", p=P),
                in_=atab_sb[:, :TF * 2 * H].rearrange("p (t h) -> p t h",
                                                      h=2 * H))
            if LROWS:
                nc.sync.dma_start(
                    out=atab_t.ap()[TF * P:SH, :],
                    in_=atab_sb[:LROWS, TF * 2 * H:T * 2 * H])
            if LROWS:
                nc.sync.dma_start(
                    out=h_t.ap()[TF * P:SH, :],
                    in_=h_sb[:LROWS, TF * HC:T * HC])
    nc.compile()

    in_maps = []
    for c in range(NCORES):
        ea_sl = np.zeros(EASH, dtype=np.float32)
        lo, hi = c * (E // NCORES), (c + 1) * (E // NCORES)
        if c == NCORES - 1:
            hi = E
        seg = ea_all[lo:hi]
        ea_sl[:seg.shape[0]] = seg
        in_maps.append({
            "xT_sh": np.ascontiguousarray(xT_bf[:, c * SH:(c + 1) * SH]),
            "W": W, "W_bf": W_bf, "asrc_rep": asrc_rep, "adst_rep": adst_rep,
            "wedge": W_edge.reshape(1, HC).astype(np.float32),
            "aedge": att_edge.reshape(1, HC).astype(np.float32),
            "ea_sh": np.ascontiguousarray(ea_sl.reshape(EASH // P, P).T).astype(NPBF),
            "ones_col": ones_col,
        })
    resA = _run(nc, in_maps, ["a_tab", "h_out", "we_out", "ea_sum"])
    LAST_RESULTS.append(resA)

    a_tab = np.concatenate([r["a_tab"] for r in resA.results], axis=0)
    h_all = np.concatenate([np.asarray(r["h_out"]) for r in resA.results],
                           axis=0)  # [N, HC] bf16, permuted channel order
    we = resA.results[0]["we_out"][0].astype(np.float32)
    ea_mean = float(sum(float(r["ea_sum"][0, 0]) for r in resA.results)) / E

    # ------------------------------------------------------------------
    # Host: edges (+self-loops) -> per-core per-dst-tile chunk slots
    # ------------------------------------------------------------------
    loops = np.arange(N, dtype=np.int64)
    src_x = np.concatenate([src_all, loops])
    dst_x = np.concatenate([dst_all, loops])
    ea_x = np.concatenate([ea_all, np.full(N, ea_mean, dtype=np.float32)])

    per_core = []
    for c in range(NCORES):
        m = (dst_x >= c * SH) & (dst_x < (c + 1) * SH)
        s, dd, ee = src_x[m], dst_x[m] - c * SH, ea_x[m]
        order = np.argsort(dd, kind="stable")
        s, dd, ee = s[order], dd[order], ee[order]
        tb = dd // P
        bounds = np.searchsorted(tb, np.arange(T + 1))
        per_core.append((s, dd, ee, bounds))

    nch = [max(int(per_core[c][3][t + 1] - per_core[c][3][t] + P - 1) // P
               for c in range(NCORES)) for t in range(T)]
    hks = [min(t_n, int(round(R_HE * t_n))) for t_n in nch]
    xks = [t_n - h_k for t_n, h_k in zip(nch, hks)]
    NCH = sum(nch)
    NCHE = sum(hks)
    NCHX = sum(xks)
    TOTMAX = max(nch)
    offs = np.concatenate([[0], np.cumsum(nch)]).astype(np.int64)
    offse = np.concatenate([[0], np.cumsum(hks)]).astype(np.int64)
    offsx = np.concatenate([[0], np.cumsum(xks)]).astype(np.int64)

    FLD = 2 * H + 1  # packed per-edge fields: asrc(8) adst(8) ea(1)
    # merged bf16 stream layout per tile: [he hk*256 | xe xk*128 | pk tot*FLD]
    mgw = [h_k * HC + x_k * P + t_n * FLD
           for t_n, h_k, x_k in zip(nch, hks, xks)]
    moff = np.concatenate([[0], np.cumsum(mgw)]).astype(np.int64)
    MGW = int(moff[-1])
    MGMAX = max(mgw)
    core_inputs = []
    for c in range(NCORES):
        s, dd, ee, bounds = per_core[c]
        gsrc = np.zeros(NCH * P, dtype=np.int64)
        pad = np.ones(NCH * P, dtype=bool)
        drel_all = np.zeros(NCH * P, dtype=np.int64)
        for t in range(T):
            lo, hi = int(bounds[t]), int(bounds[t + 1])
            n = hi - lo
            base = int(offs[t]) * P
            gsrc[base:base + n] = s[lo:hi]
            pad[base:base + n] = False
            drel_all[base:base + n] = dd[lo:hi] - t * P
        mg = np.zeros((P, MGW), dtype=NPBF)
        for t in range(T):
            tot, hk, xk = nch[t], hks[t], xks[t]
            b0 = int(offs[t]) * P
            m0 = int(moff[t])
            # he chunks: slots [b0, b0+hk*128)
            if hk:
                sl = slice(b0, b0 + hk * P)
                blk = h_all[gsrc[sl]]              # [hk*128, 256]
                blk[pad[sl]] = NPBF(0)
                mg[:, m0:m0 + hk * HC] = (
                    blk.reshape(hk, P, HC).transpose(1, 0, 2)
                    .reshape(P, hk * HC))
            # xe chunks: slots [b0+hk*128, b0+tot*128)
            if xk:
                sl = slice(b0 + hk * P, b0 + tot * P)
                xblk = xT_bf[:, gsrc[sl]]          # [128, xk*128]
                xblk[:, pad[sl]] = NPBF(0)
                mg[:, m0 + hk * HC:m0 + hk * HC + xk * P] = xblk
            # pk fields
            lo, hi = int(bounds[t]), int(bounds[t + 1])
            n = hi - lo
            sl = slice(lo, hi)

            def expand(vals, w):
                buf = np.zeros((tot * P, w), dtype=np.float32)
                buf[:n] = vals.reshape(n, w)
                return (buf.reshape(tot, P, w).transpose(1, 0, 2)
                        .reshape(P, tot * w))

            p0 = m0 + hk * HC + xk * P
            mg[:, p0:p0 + tot * H] = expand(a_tab[s[sl], 0:H], H)
            mg[:, p0 + tot * H:p0 + 2 * tot * H] = expand(
                a_tab[c * SH + dd[sl], H:2 * H], H)
            mg[:, p0 + 2 * tot * H:p0 + FLD * tot] = expand(ee[sl], 1)
        # one-hot dst-selection matrices (fp8), precomputed host-side
        soh = np.zeros((NCH, P, P), dtype=NPF8)
        real = ~pad
        slot = np.nonzero(real)[0]
        soh[slot // P, slot % P, drel_all[real]] = NPF8(1)
        soh = np.ascontiguousarray(
            soh.transpose(1, 0, 2).reshape(P, NCH * P))
        core_inputs.append(dict(mg=mg, soh=soh))

    we_tiled = np.ascontiguousarray(
        np.tile(we.reshape(1, 1, H), (P, TOTMAX, 1))
        .reshape(P, TOTMAX * H)).astype(NPBF)

    # ------------------------------------------------------------------
    # Launch B
    # ------------------------------------------------------------------
    nc = bacc.Bacc("TRN2", target_bir_lowering=False, debug=False)
    mg_t = nc.dram_tensor("mg", [P, MGW], BF16, kind="ExternalInput")
    soh_t = nc.dram_tensor("soh", [P, NCH * P], FP8, kind="ExternalInput")
    W_t = nc.dram_tensor("W", [P, HC], BF16, kind="ExternalInput")
    onesb_t = nc.dram_tensor("ones_bf", [P, 1], BF16, kind="ExternalInput")
    wet_t = nc.dram_tensor("we_tiled", [P, TOTMAX * H], BF16,
                           kind="ExternalInput")
    opre_t = nc.dram_tensor("out_pre", [SH, HC], BF16, kind="ExternalOutput")
    stats_t = nc.dram_tensor("stats", [1, 2 * HC], F32, kind="ExternalOutput")

    with tile.TileContext(nc) as tc:
        with tc.tile_pool(name="const", bufs=1) as cpool:
            w_sb = cpool.tile([P, HC], BF16, tag="w")
            nc.sync.dma_start(out=w_sb[:], in_=W_t.ap())
            on_sb = cpool.tile([P, 1], BF16, tag="ones")
            nc.sync.dma_start(out=on_sb[:], in_=onesb_t.ap())
            wet_sb = cpool.tile([P, TOTMAX * H], BF16, tag="wet")
            nc.sync.dma_start(out=wet_sb[:], in_=wet_t.ap())

            with tc.tile_pool(name="mg", bufs=3) as mgpool, \
                 tc.tile_pool(name="mb", bufs=2) as mpool, \
                 tc.tile_pool(name="s", bufs=3) as spool, \
                 tc.tile_pool(name="fin", bufs=3) as fpool, \
                 tc.tile_pool(name="hp", bufs=2, space="PSUM") as hpp, \
                 tc.tile_pool(name="acc", bufs=2, space="PSUM") as apool, \
                 tc.tile_pool(name="stp", bufs=1, space="PSUM") as stpool:
                stats_ps = stpool.tile([1, 2 * HC], F32, tag="stats")
                for t in range(T):
                    rows = min(P, SH - t * P)
                    tot, hk, xk = nch[t], hks[t], xks[t]
                    off = int(offs[t])
                    m0 = int(moff[t])
                    mg_sb = mgpool.tile([P, MGMAX], BF16, tag="mg")
                    nc.sync.dma_start(out=mg_sb[:, :mgw[t]],
                                      in_=mg_t.ap()[:, m0:m0 + mgw[t]])
                    he_v = mg_sb[:, 0:hk * HC]
                    xe_v = mg_sb[:, hk * HC:hk * HC + xk * P]
                    pk_v = mg_sb[:, hk * HC + xk * P:
                                  hk * HC + xk * P + tot * FLD]
                    s_sb = spool.tile([P, TOTMAX * P], FP8, tag="S")
                    nc.sync.dma_start(out=s_sb[:, :tot * P],
                                      in_=soh_t.ap()[:, off * P:(off + tot) * P])
                    a1 = pk_v[:, 0:tot * H]
                    a2 = pk_v[:, tot * H:2 * tot * H]
                    eav = pk_v[:, 2 * tot * H:2 * tot * H + tot]
                    # alpha = asrc + adst + ea*we ; w = exp(leaky_relu(alpha))
                    nc.vector.tensor_add(a1, a1, a2)
                    nc.vector.tensor_mul(
                        a2.rearrange("p (k h) -> p k h", h=H),
                        eav.to_broadcast([P, tot, H]),
                        wet_sb[:, :tot * H].rearrange("p (k h) -> p k h", h=H))
                    nc.vector.tensor_add(a1, a1, a2)
                    nc.scalar.activation(a2, a1,
                                         mybir.ActivationFunctionType.Relu,
                                         scale=-float(1.0 - NEG_SLOPE))
                    nc.vector.tensor_add(a1, a1, a2)
                    mb = mpool.tile([P, TOTMAX * MBW], BF16, tag="mb")
                    mbv = mb[:, :tot * MBW].rearrange("p (k e) -> p k e", e=MBW)
                    nc.scalar.activation(mbv[:, :, HC:MBW],
                                         a1.rearrange("p (k h) -> p k h", h=H),
                                         mybir.ActivationFunctionType.Exp)
                    acc = apool.tile([P, MBW], F32, tag="acc")

                    def ekb_for(k0, run):
                        return (mb[:, k0 * MBW:(k0 + run) * MBW]
                                .rearrange("p (k e) -> p k e", e=MBW)
                                [:, :, HC:MBW]
                                .rearrange("p k (o h) -> p k o h", o=1)
                                .to_broadcast([P, run, C, H]))

                    def mb_msg(k0, run):
                        return (mb[:, k0 * MBW:(k0 + run) * MBW]
                                .rearrange("p (k e) -> p k e", e=MBW)
                                [:, :, 0:HC]
                                .rearrange("p k (c h) -> p k c h", h=H))

                    def scat(k):
                        nc.tensor.matmul(
                            acc[:], lhsT=s_sb[:, k * P:(k + 1) * P],
                            rhs=mb[:, k * MBW:(k + 1) * MBW],
                            start=(k == 0), stop=(k == tot - 1))

                    # he chunks: k in [0, hk) — mul straight from SBUF
                    for k0 in range(0, hk, 8):
                        run = min(8, hk - k0)
                        nc.vector.tensor_mul(
                            mb_msg(k0, run),
                            he_v[:, k0 * HC:(k0 + run) * HC].rearrange(
                                "p (k c h) -> p k c h", c=C, h=H),
                            ekb_for(k0, run))
                        for j in range(run):
                            scat(k0 + j)
                    # xe chunks: k in [hk, tot) — project on PE, then mul
                    for k0 in range(hk, tot, 4):
                        run = min(4, tot - k0)
                        hps = hpp.tile([P, 4 * HC], F32, tag="hps")
                        for j in range(run):
                            nc.tensor.matmul(
                                hps[:, j * HC:(j + 1) * HC],
                                lhsT=xe_v[:, (k0 + j - hk) * P:
                                          (k0 + j - hk + 1) * P],
                                rhs=w_sb[:], start=True, stop=True)
                        nc.vector.tensor_mul(
                            mb_msg(k0, run),
                            hps[:, :run * HC].rearrange(
                                "p (k c h) -> p k c h", c=C, h=H),
                            ekb_for(k0, run))
                        for j in range(run):
                            scat(k0 + j)
                    # finalize tile: normalize + stats
                    den = fpool.tile([P, H], F32, tag="den")
                    nc.scalar.activation(den[:rows], acc[:rows, HC:MBW],
                                         mybir.ActivationFunctionType.Copy)
                    rec = fpool.tile([P, H], F32, tag="rec")
                    nc.vector.reciprocal(rec[:rows], den[:rows])
                    opsq = fpool.tile([P, 2 * HC], BF16, tag="opsq")
                    nc.vector.tensor_mul(
                        opsq[:rows, :HC].rearrange("p (c h) -> p c h", h=H),
                        acc[:rows, :HC].rearrange("p (c h) -> p c h", h=H),
                        rec[0:rows, :].rearrange("p (o h) -> p o h",
                                                 o=1).to_broadcast(
                            [rows, C, H]))
                    nc.scalar.activation(opsq[:rows, HC:], opsq[:rows, :HC],
                                         mybir.ActivationFunctionType.Square)
                    nc.tensor.matmul(stats_ps[:, :], lhsT=on_sb[:rows, :],
                                     rhs=opsq[:rows, :], start=(t == 0),
                                     stop=(t == T - 1))
                    nc.sync.dma_start(out=opre_t.ap()[t * P:t * P + rows, :],
                                      in_=opsq[:rows, :HC])
                st_sb = fpool.tile([1, 2 * HC], F32, tag="stsb")
                nc.vector.tensor_copy(st_sb[:], stats_ps[:])
                nc.sync.dma_start(out=stats_t.ap(), in_=st_sb[:])
    nc.compile()

    in_maps = []
    for c in range(NCORES):
        ci = core_inputs[c]
        in_maps.append({
            "mg": ci["mg"], "soh": ci["soh"],
            "W": W_bf, "ones_bf": ones_bf, "we_tiled": we_tiled,
        })
    resB = _run(nc, in_maps, ["out_pre", "stats"])
    LAST_RESULTS.append(resB)

    out_pre = np.concatenate([np.asarray(r["out_pre"])
                              for r in resB.results], axis=0)
    stats = np.stack([np.asarray(r["stats"][0], dtype=np.float64)
                      for r in resB.results]).sum(axis=0).astype(np.float32)
    sums_col = np.ascontiguousarray(
        np.stack([stats[:HC], stats[HC:]], axis=1))  # [HC, 2]

    # ------------------------------------------------------------------
    # Launch C: batchnorm + ELU (transposed layout)
    # ------------------------------------------------------------------
    opT = np.ascontiguousarray(out_pre.reshape(NCORES, SH, HC)
                               .transpose(0, 2, 1))  # [8, HC, SH] bf16
    nc = bacc.Bacc("TRN2", target_bir_lowering=False, debug=False)
    opT_t = nc.dram_tensor("opT", [HC, SH], BF16, kind="ExternalInput")
    sums_t = nc.dram_tensor("sums_col", [HC, 2], F32, kind="ExternalInput")
    gam_t = nc.dram_tensor("gamma_col", [HC, 1], F32, kind="ExternalInput")
    bet_t = nc.dram_tensor("beta_col", [HC, 1], F32, kind="ExternalInput")
    outT_t = nc.dram_tensor("outT", [HC, SH], F32, kind="ExternalOutput")

    CT = HC // P
    with tile.TileContext(nc) as tc:
        with tc.tile_pool(name="sbuf", bufs=2) as pool:
            for ct in range(CT):
                sm = pool.tile([P, 2], F32, tag="sm")
                nc.sync.dma_start(out=sm[:], in_=sums_t.ap()[ct * P:(ct + 1) * P, :])
                gm = pool.tile([P, 1], F32, tag="gm")
                nc.sync.dma_start(out=gm[:], in_=gam_t.ap()[ct * P:(ct + 1) * P, :])
                bt = pool.tile([P, 1], F32, tag="bt")
                nc.sync.dma_start(out=bt[:], in_=bet_t.ap()[ct * P:(ct + 1) * P, :])
                mean = pool.tile([P, 1], F32, tag="mean")
                nc.vector.tensor_scalar_mul(mean[:], sm[:, 0:1], 1.0 / N)
                ex2 = pool.tile([P, 1], F32, tag="ex2")
                nc.vector.tensor_scalar_mul(ex2[:], sm[:, 1:2], 1.0 / N)
                msq = pool.tile([P, 1], F32, tag="msq")
                nc.vector.tensor_mul(msq[:], mean[:], mean[:])
                var = pool.tile([P, 1], F32, tag="var")
                nc.vector.tensor_sub(var[:], ex2[:], msq[:])
                nc.vector.tensor_scalar_add(var[:], var[:], float(BN_EPS))
                sd = pool.tile([P, 1], F32, tag="sd")
                nc.scalar.activation(sd[:], var[:],
                                     mybir.ActivationFunctionType.Sqrt)
                inv = pool.tile([P, 1], F32, tag="inv")
                nc.vector.reciprocal(inv[:], sd[:])
                scl = pool.tile([P, 1], F32, tag="scl")
                nc.vector.tensor_mul(scl[:], inv[:], gm[:])
                sh1 = pool.tile([P, 1], F32, tag="sh1")
                nc.vector.tensor_mul(sh1[:], mean[:], scl[:])
                shf = pool.tile([P, 1], F32, tag="shf")
                nc.vector.tensor_sub(shf[:], bt[:], sh1[:])
                NS = 5  # column splits for pipelining
                CW = (SH + NS - 1) // NS
                for cs in range(NS):
                    c0 = cs * CW
                    cw = min(CW, SH - c0)
                    xt_ = pool.tile([P, CW], BF16, tag="xt")
                    nc.sync.dma_start(
                        out=xt_[:, :cw],
                        in_=opT_t.ap()[ct * P:(ct + 1) * P, c0:c0 + cw])
                    # elu(y) = relu(y) + min(exp(y), 1) - 1,  y = scl*x + shf
                    r = pool.tile([P, CW], F32, tag="r")
                    nc.scalar.activation(r[:, :cw], xt_[:, :cw],
                                         mybir.ActivationFunctionType.Relu,
                                         bias=shf[:], scale=scl[:])
                    e = pool.tile([P, CW], F32, tag="e")
                    nc.scalar.activation(e[:, :cw], xt_[:, :cw],
                                         mybir.ActivationFunctionType.Exp,
                                         bias=shf[:], scale=scl[:])
                    m1 = pool.tile([P, CW], F32, tag="m1")
                    nc.vector.tensor_scalar_min(m1[:, :cw], e[:, :cw], 1.0)
                    out_f = pool.tile([P, CW], F32, tag="outf")
                    nc.vector.scalar_tensor_tensor(
                        out_f[:, :cw], m1[:, :cw], -1.0, r[:, :cw],
                        op0=mybir.AluOpType.add, op1=mybir.AluOpType.add)
                    nc.sync.dma_start(
                        out=outT_t.ap()[ct * P:(ct + 1) * P, c0:c0 + cw],
                        in_=out_f[:, :cw])
    nc.compile()

    in_maps = [{
        "opT": np.ascontiguousarray(opT[c]),
        "sums_col": sums_col,
        "gamma_col": gamma[old_of_new].reshape(HC, 1),
        "beta_col": beta[old_of_new].reshape(HC, 1),
    } for c in range(NCORES)]
    resC = _run(nc, in_maps, ["outT"])
    LAST_RESULTS.append(resC)

    outp = np.concatenate(
        [np.asarray(r["outT"]).T for r in resC.results], axis=0)  # [N, HC]
    out = np.empty_like(outp)
    out[:, old_of_new] = outp  # undo the [c,h] channel permutation
    return np.ascontiguousarray(out.astype(np.float32))
